# revision 7
# baseline (speedup 1.0000x reference)
"""Trainium2 Bass kernel for nn_CombineLoss (focal + dice + edge loss), v2.

Sharding: data-parallel over batch B=8 -> one image per NeuronCore.

v2 strategy (vs v1 baseline at 137us):
  - Inputs staged to HBM as float16 (host-side cast): preds 8MB + diss 4MB
    + target 2MB = 14MB/core, halving the DMA roofline.
  - 7x7 circular conv via rank-3 row-pattern decomposition:
      mask cols: dx=0 -> 7-tap dy band (B7C, with -29 diag fold),
      |dx|in{1,2} -> 5-tap dy band applied to U = sum of 4 col-shifts,
      |dx|=3 -> identity applied to V = sum of 2 col-shifts.
    Cross-tile dy halo handled by ONE extra matmul per tile over a packed
    [10,W] buffer of neighbor boundary rows (sbuf->sbuf DMA packing).
    4 wide (1024-free) matmuls per tile instead of ~32.
  - Sums: focal sums ride free on ACT accum_out; dice/edge sums via
    ones-stationary matmuls reducing product planes over partitions into
    PSUM slots (partition bases 0/32/64) accumulated across all 8 tiles.
  - Per-element products batched into 3-plane concat DVE multiplies.
  - GpSimd takes sgn/V/|psum| to offload DVE.
  - ACT ops batched 4 tiles per function to amortize act-table loads.

Math per head (C=2 softmax heads): d = x1-x0, s = (2t-1)*d, pt = sigmoid(s),
lp = ln(pt+eps); focal = mean(-lp); edge = mean(-lp*at), at = |t-ave7x7|;
dice from I = sum(pt*t), sum(pt) (sum p1 = n0 + 2I - sum(pt)).
Diss head: ptd = t ? dt1 : dt0 (copy_predicated), p1 = dt1.
"""

import numpy as np

N_HEADS_PRED = 2
B = 8
C = 2
H = 1024
W = 1024
P = 128
NT = H // P          # 8 row tiles
TBW = 1032           # padded tb width (3 left, 5 right)
EPS = 1e-10
N_CORES = 8
NSLOT = NT * 3       # ACT accum slots: per tile [S, F01, Fd]
NPS = 7              # psum stat rows: spt0, i0, i1, i_d, sd1, e01, ed

_CACHE = {}


def _ensure_path():
    try:
        import concourse  # noqa: F401
    except ImportError:
        import sys
        for p in ("/opt/trn_rl_repo", "/root/.axon_site/_ro/trn_rl_repo"):
            if p not in sys.path:
                sys.path.insert(0, p)


def _make_bands():
    """[3,128,128] f16 stationaries (matmul: out[p,n] = sum_m stat[m,p]*mov[m,n]).

    B7C: |dy|<=3 band with -29 on the diagonal (folds -29*t into the conv).
    B5C: |dy|<=2 band (applied to U).
    I:   identity (applied to V).
    All entries small ints -> exact in f16.
    """
    idx = np.arange(P)
    dy = idx[:, None] - idx[None, :]
    b7c = (np.abs(dy) <= 3).astype(np.float32) - 29.0 * (dy == 0)
    b5c = (np.abs(dy) <= 2).astype(np.float32)
    i128 = (dy == 0).astype(np.float32)
    return np.stack([b7c, b5c, i128]).astype(np.float16)


def _make_shalo():
    """[16,128] f16 packed-halo stationary.

    Packed moving rows (per tile i):
      0-2: tb[i-1] rows 125+j (global -3+j) -> B7 taps: out p <= j
      3-4: U[i-1] rows 126+j (global -2+j) -> B5 taps: out p <= j
      5-7: tb[i+1] rows j (global 128+j)   -> B7 taps: out p >= 125+j
      8-9: U[i+1] rows j (global 128+j)    -> B5 taps: out p >= 126+j
    Rows 10-15 unused (zero).
    """
    s = np.zeros((16, P), np.float32)
    for j in range(3):
        s[j, : j + 1] = 1.0          # T_prev row -3+j -> out p in [0, j]
    for j in range(2):
        s[3 + j, : j + 1] = 1.0      # U_prev row -2+j -> out p in [0, j]
    for j in range(3):
        s[5 + j, 125 + j:] = 1.0     # T_next row 128+j -> out p in [125+j, 127]
    for j in range(2):
        s[8 + j, 126 + j:] = 1.0     # U_next row 128+j -> out p in [126+j, 127]
    return s.astype(np.float16)


def _build_nc():
    _ensure_path()
    import concourse.mybir as mybir
    from concourse import bacc
    from concourse.tile import TileContext

    f32 = mybir.dt.float32
    f16 = mybir.dt.float16
    Alu = mybir.AluOpType
    Act = mybir.ActivationFunctionType

    nc = bacc.Bacc()
    preds = nc.dram_tensor("preds", [N_HEADS_PRED, C, H, W], f16,
                           kind="ExternalInput")
    diss = nc.dram_tensor("diss", [C, H, W], f16, kind="ExternalInput")
    target = nc.dram_tensor("target", [H, W], f16, kind="ExternalInput")
    bands = nc.dram_tensor("bands", [3, P, P], f16, kind="ExternalInput")
    shalo = nc.dram_tensor("shalo", [16, P], f16, kind="ExternalInput")
    stats_out = nc.dram_tensor("stats", [P, NSLOT], f32, kind="ExternalOutput")
    pstats_out = nc.dram_tensor("pstats", [NPS, 512], f32,
                                kind="ExternalOutput")

    KACT = 4  # ACT function-batching depth (tiles)

    with TileContext(nc) as tc:
        with (
            tc.tile_pool(name="const", bufs=1) as constp,
            tc.tile_pool(name="res", bufs=1) as resp,
            tc.tile_pool(name="xin", bufs=2) as xinp,
            tc.tile_pool(name="din", bufs=2) as dinp,
            tc.tile_pool(name="early", bufs=2) as earlyp,
            tc.tile_pool(name="workp", bufs=KACT + 1) as workp,
            tc.tile_pool(name="scatp", bufs=KACT) as scatp,
            tc.tile_pool(name="lpp", bufs=KACT) as lpp,
            tc.tile_pool(name="prod", bufs=2) as prodp,
            tc.tile_pool(name="cps", bufs=2, space="PSUM") as cpsp,
            tc.tile_pool(name="sps", bufs=1, space="PSUM") as spsp,
        ):
            b7c = constp.tile([P, P], f16)
            b5c = constp.tile([P, P], f16)
            i128 = constp.tile([P, P], f16)
            nc.sync.dma_start(out=b7c, in_=bands[0])
            nc.sync.dma_start(out=b5c, in_=bands[1])
            nc.sync.dma_start(out=i128, in_=bands[2])
            sh = constp.tile([16, P], f16)
            nc.sync.dma_start(out=sh, in_=shalo[:, :])
            ones = constp.tile([P, 1], f16)
            nc.gpsimd.memset(ones[:], 1.0)
            eps_t = constp.tile([P, 1], f32)
            nc.gpsimd.memset(eps_t[:], EPS)
            stats_sb = constp.tile([P, NSLOT], f32)

            tb_all = resp.tile([P, NT, TBW], f16)
            u_all = resp.tile([P, NT, W], f16)
            packed = resp.tile([16, NT, W], f16)
            nc.gpsimd.memset(tb_all[:], 0.0)
            nc.gpsimd.memset(packed[:], 0.0)

            # 3 psum stat tiles; slots at partition bases 0/32/64.
            psA = spsp.tile([P, 512], f32)  # 0: spt0, 32: i0, 64: i1
            psB = spsp.tile([P, 512], f32)  # 0: i_d, 32: sd1, 64: e01
            psC = spsp.tile([P, 512], f32)  # 0: ed

            # ---- phase A: target tiles, U/V pre-sums -----------------------
            for i in range(NT):
                tb_i = tb_all[:, i, 3:3 + W]
                nc.sync.dma_start(out=tb_i, in_=target[i * P:(i + 1) * P, :])
                t0 = tb_all[:, i, :]
                u1 = u_all[:, i, :]
                nc.vector.tensor_tensor(u1, t0[:, 2:2 + W], t0[:, 4:4 + W],
                                        Alu.add)
                nc.vector.tensor_tensor(u1, u1, t0[:, 1:1 + W], Alu.add)
                nc.vector.tensor_tensor(u1, u1, t0[:, 5:5 + W], Alu.add)

            # ---- phase A2: packed halo rows (sbuf->sbuf) -------------------
            for i in range(NT):
                if i > 0:
                    nc.sync.dma_start(out=packed[0:3, i, :],
                                      in_=tb_all[125:128, i - 1, 3:3 + W])
                    nc.sync.dma_start(out=packed[3:5, i, :],
                                      in_=u_all[126:128, i - 1, :])
                if i < NT - 1:
                    nc.sync.dma_start(out=packed[5:8, i, :],
                                      in_=tb_all[0:3, i + 1, 3:3 + W])
                    nc.sync.dma_start(out=packed[8:10, i, :],
                                      in_=u_all[0:2, i + 1, :])

            # ---- phase B: per-tile compute ---------------------------------
            # ACT ops are emitted in KACT-tile function-major batches with
            # alternating order so act-table loads amortize to ~3 total.
            sgn_t = [None] * NT
            at_t = [None] * NT
            scat_t = [None] * NT
            workB_t = [None] * NT
            d0t_t = [None] * NT
            ptd_t = [None] * NT
            lpB_t = [None] * NT

            def stage1(i):
                # conv matmuls -> psum = 29*ave - 29*t ; at = |psum| (f16)
                t0 = tb_all[:, i, :]
                v = earlyp.tile([P, 1, W], f16, tag="v")
                nc.gpsimd.tensor_tensor(v[:, 0, :], t0[:, 0:0 + W],
                                        t0[:, 6:6 + W], Alu.add)
                psum = cpsp.tile([P, W], f32, tag="cpsum")
                for h in range(2):
                    c0 = h * 512
                    nc.tensor.matmul(psum[:, c0:c0 + 512], b7c,
                                     tb_all[:, i, 3 + c0:3 + c0 + 512],
                                     start=True, stop=False)
                    nc.tensor.matmul(psum[:, c0:c0 + 512], b5c,
                                     u_all[:, i, c0:c0 + 512],
                                     start=False, stop=False)
                    nc.tensor.matmul(psum[:, c0:c0 + 512], i128,
                                     v[:, 0, c0:c0 + 512],
                                     start=False, stop=False)
                    nc.tensor.matmul(psum[:, c0:c0 + 512], sh,
                                     packed[:, i, c0:c0 + 512],
                                     start=False, stop=True)
                at = earlyp.tile([P, 1, W], f16, tag="at")
                nc.scalar.activation(at[:, 0, :], psum, Act.Abs)
                at_t[i] = at

                sgn = earlyp.tile([P, 1, W], f16, tag="sgn")
                nc.gpsimd.tensor_scalar(sgn[:, 0, :], tb_all[:, i, 3:3 + W],
                                        2.0, -1.0, Alu.mult, Alu.add)
                sgn_t[i] = sgn

                xt = xinp.tile([P, 2, C, W], f16, tag="xt")
                nc.sync.dma_start(
                    out=xt, in_=preds[:, :, i * P:(i + 1) * P, :].rearrange(
                        "n c h w -> h n c w"))
                d0t = dinp.tile([P, W], f16, tag="d0t")
                nc.sync.dma_start(out=d0t, in_=diss[0, i * P:(i + 1) * P, :])
                workB = workp.tile([P, 3, W], f16, tag="workB")
                nc.sync.dma_start(out=workB[:, 2, :],
                                  in_=diss[1, i * P:(i + 1) * P, :])
                workB_t[i] = workB
                d0t_t[i] = d0t

                dcat = earlyp.tile([P, 2, W], f16, tag="dcat")
                nc.vector.tensor_tensor(dcat, xt[:, :, 1, :], xt[:, :, 0, :],
                                        Alu.subtract)
                scat = scatp.tile([P, 2, W], f16, tag="scat")
                nc.vector.tensor_tensor(
                    scat, dcat, sgn[:, 0:1, :].broadcast_to([P, 2, W]),
                    Alu.mult)
                scat_t[i] = scat

                # diss select: ptd = t ? dt1 : dt0
                ptd = scatp.tile([P, W], f16, tag="ptd")
                nc.vector.tensor_copy(ptd, d0t)
                nc.vector.copy_predicated(
                    ptd, tb_all[:, i, 3:3 + W].bitcast(mybir.dt.uint16),
                    workB[:, 2, :])
                ptd_t[i] = ptd

            def act_sig(i):
                base = i * 3
                nc.scalar.activation(workB_t[i][:, 0:2, :], scat_t[i],
                                     Act.Sigmoid,
                                     accum_out=stats_sb[:, base:base + 1])

            def act_ln(i):
                base = i * 3
                lpB = lpp.tile([P, 3, W], f16, tag="lpB")
                nc.scalar.activation(lpB[:, 0:2, :], workB_t[i][:, 0:2, :],
                                     Act.Ln, bias=eps_t[:, 0:1],
                                     accum_out=stats_sb[:, base + 1:base + 2])
                nc.scalar.activation(lpB[:, 2, :], ptd_t[i], Act.Ln,
                                     bias=eps_t[:, 0:1],
                                     accum_out=stats_sb[:, base + 2:base + 3])
                lpB_t[i] = lpB

            def stage2(i):
                # products and ones-matmul reductions
                tb_b = tb_all[:, i:i + 1, 3:3 + W].broadcast_to([P, 3, W])
                at_b = at_t[i].broadcast_to([P, 3, W])
                maskO = prodp.tile([P, 3, W], f16, tag="maskO")
                nc.vector.tensor_tensor(maskO, workB_t[i], tb_b, Alu.mult)
                atO = prodp.tile([P, 3, W], f16, tag="atO")
                nc.vector.tensor_tensor(atO, lpB_t[i], at_b, Alu.mult)

                st8 = (i == 0)
                sp8 = (i == NT - 1)
                for h in range(2):
                    c0 = h * 512
                    # psA: spt0 (pt0 plain), i0 (ptt0), i1 (ptt1)
                    nc.tensor.matmul(psA[0:1, :], ones,
                                     workB_t[i][:, 0, c0:c0 + 512],
                                     start=(st8 and h == 0),
                                     stop=(sp8 and h == 1))
                    nc.tensor.matmul(psA[32:33, :], ones,
                                     maskO[:, 0, c0:c0 + 512],
                                     start=(st8 and h == 0),
                                     stop=(sp8 and h == 1))
                    nc.tensor.matmul(psA[64:65, :], ones,
                                     maskO[:, 1, c0:c0 + 512],
                                     start=(st8 and h == 0),
                                     stop=(sp8 and h == 1))
                    # psB: i_d (dtt), sd1 (dt1 plain), e01 (lp0at+lp1at)
                    nc.tensor.matmul(psB[0:1, :], ones,
                                     maskO[:, 2, c0:c0 + 512],
                                     start=(st8 and h == 0),
                                     stop=(sp8 and h == 1))
                    nc.tensor.matmul(psB[32:33, :], ones,
                                     workB_t[i][:, 2, c0:c0 + 512],
                                     start=(st8 and h == 0),
                                     stop=(sp8 and h == 1))
                    nc.tensor.matmul(psB[64:65, :], ones,
                                     atO[:, 0, c0:c0 + 512],
                                     start=(st8 and h == 0), stop=False)
                    nc.tensor.matmul(psB[64:65, :], ones,
                                     atO[:, 1, c0:c0 + 512],
                                     start=False, stop=(sp8 and h == 1))
                    # psC: ed (lpd*at)
                    nc.tensor.matmul(psC[0:1, :], ones,
                                     atO[:, 2, c0:c0 + 512],
                                     start=(st8 and h == 0),
                                     stop=(sp8 and h == 1))

            # Emit: stage1 for all tiles in batch, then function-major ACT,
            # then stage2. Alternate sig/ln order between batches.
            for b0 in range(0, NT, KACT):
                tiles = list(range(b0, min(b0 + KACT, NT)))
                for i in tiles:
                    stage1(i)
                for i in tiles:
                    act_sig(i)
                for i in tiles:
                    act_ln(i)
                for i in tiles:
                    stage2(i)

            # copy psum stat rows -> sbuf, DMA out
            pout = constp.tile([P, 3, 512], f32)
            nc.vector.tensor_copy(pout[:, 0, :], psA)
            nc.vector.tensor_copy(pout[:, 1, :], psB)
            nc.vector.tensor_copy(pout[0:1, 2, :], psC[0:1, :])
            nc.sync.dma_start(out=pstats_out[0:1, :], in_=pout[0:1, 0, :])
            nc.sync.dma_start(out=pstats_out[1:2, :], in_=pout[32:33, 0, :])
            nc.sync.dma_start(out=pstats_out[2:3, :], in_=pout[64:65, 0, :])
            nc.sync.dma_start(out=pstats_out[3:4, :], in_=pout[0:1, 1, :])
            nc.sync.dma_start(out=pstats_out[4:5, :], in_=pout[32:33, 1, :])
            nc.sync.dma_start(out=pstats_out[5:6, :], in_=pout[64:65, 1, :])
            nc.sync.dma_start(out=pstats_out[6:7, :], in_=pout[0:1, 2, :])
            nc.sync.dma_start(out=stats_out[:], in_=stats_sb[:])

    nc.finalize()
    return nc


def get_program():
    if "nc" not in _CACHE:
        _CACHE["nc"] = _build_nc()
    return _CACHE["nc"]


def make_in_maps(predictions, Diss, target):
    bands = _make_bands()
    sh = _make_shalo()
    in_maps = []
    for c in range(N_CORES):
        in_maps.append({
            "preds": np.ascontiguousarray(predictions[:, c]).astype(np.float16),
            "diss": np.ascontiguousarray(Diss[0, c]).astype(np.float16),
            "target": target[c].astype(np.float16),
            "bands": bands,
            "shalo": sh,
        })
    return in_maps


def assemble(stats_list, pstats_list, sigma, diff, target):
    """Combine per-core stats into the scalar loss (float64 on host)."""
    HW = float(H * W)
    sig2 = np.asarray(sigma, np.float64) ** 2
    st_host = np.asarray(target).reshape(N_CORES, -1).sum(axis=1).astype(
        np.float64)

    focal_total = 0.0
    edge_total = 0.0
    dice_ratio = np.zeros(3, np.float64)
    for c in range(N_CORES):
        g = stats_list[c].astype(np.float64).sum(axis=0)  # [NSLOT]
        S = g[0::3].sum()      # sum(pt0)+sum(pt1)
        F01 = g[1::3].sum()    # sum(lp0)+sum(lp1)
        Fd = g[2::3].sum()     # sum(lpd)
        ps = pstats_list[c].astype(np.float64).sum(axis=1)  # [NPS]
        spt0, i0, i1, i_d, sd1, e01, ed = ps
        spt1 = S - spt0
        st = st_host[c]
        U0 = HW + 2.0 * i0 - spt0
        U1 = HW + 2.0 * i1 - spt1
        Ud = sd1 + st
        dice_ratio[0] += 2.0 * i0 / (U0 + EPS)
        dice_ratio[1] += 2.0 * i1 / (U1 + EPS)
        dice_ratio[2] += 2.0 * i_d / (Ud + EPS)
        focal_total += -(F01 + Fd)
        edge_total += -(e01 + ed) / 29.0

    denom = float(N_CORES) * HW
    loss = focal_total / denom / sig2[0]
    loss += edge_total / denom / sig2[2]
    for hh in range(3):
        loss += (1.0 - dice_ratio[hh] / float(N_CORES)) / sig2[1]
    loss += float(diff)
    loss += float(np.sum(np.log(sig2))) / 2.0
    return np.float32(loss)


def run_on_hw(predictions, Diss, target, trace=False):
    _ensure_path()
    from concourse.bass_utils import run_bass_kernel_spmd
    nc = get_program()
    in_maps = make_in_maps(predictions, Diss, target)
    res = run_bass_kernel_spmd(nc, in_maps, list(range(N_CORES)), trace=trace)
    stats_list = [r["stats"] for r in res.results]
    pstats_list = [r["pstats"] for r in res.results]
    return stats_list, pstats_list, res


def kernel(predictions, Diss, target, diff, sigma):
    predictions = np.asarray(predictions)
    Diss = np.asarray(Diss)
    target = np.asarray(target)
    stats_list, pstats_list, _ = run_on_hw(predictions, Diss, target,
                                           trace=False)
    return assemble(stats_list, pstats_list, np.asarray(sigma),
                    np.asarray(diff), target)
